# revision 1
# baseline (speedup 1.0000x reference)
"""Trainium2 Bass kernel for nn_Camada_33612414059004.

Computes, for x:[B,N,D,S], M:[N,N], w_syn:[N,D,S], b_dend:[N,D],
w_dend:[N,D], b_soma:[N]:

    xm    = einsum('bids,oi->bods', x, M)
    dend  = tanh(einsum('bnds,nds->bnd', xm, w_syn) + b_dend)
    soma  = einsum('bnd,nd->bn', dend, w_dend) + b_soma
    out   = sigmoid(soma)                                  # [B, N]

Sharding: data-parallel over batch across 8 NeuronCores (B=64 -> 8/core),
zero cross-core communication.  Per core the dominant work is the
connectivity matmul  M[o,i] @ x[i, (b,d,s)]  ([1024x1024]x[1024x1024],
2.15 GFLOP, bf16 operands / fp32 PSUM accumulate) on the TensorEngine.
The per-neuron stages run in fp32, spread so no engine exceeds the PE's
3.5us-per-o-tile matmul time: Vector multiplies the PSUM result by w_syn
and does the s/d reductions, GpSimd does the small bias-add and w_dend
multiply, Scalar does tanh / sigmoid(+b_soma).

Per-core on-chip layout: output neurons `o` on the 128 SBUF partitions
(8 o-tiles), free dim ordered (b, d, s).

Schedule: o-tiles 0-3 accumulate k-outer — per-k PE work (4 matmul pairs)
matches the per-k input DMA time, riding out the input stream; o-tiles
4-7 run k-inner one at a time, each tile's postprocess chain pipelining
against the next tile's matmuls.  x chunks load on the Sync HWDGE, mt
chunks on the Scalar HWDGE (parallel issue); all per-neuron parameters
are packed host-side into one contiguous [128, 1160] fp32 array moved by
a single DMA.
"""

import numpy as np
import ml_dtypes
from contextlib import ExitStack

import concourse.bass as bass
import concourse.mybir as mybir
import concourse.tile as tile

B, N, D, S = 64, 1024, 8, 16
NCORES = 8
BC = B // NCORES          # batches per core = 8
DS = D * S                # 128
P = 128                   # SBUF partitions
KT = N // P               # 8 contraction chunks (input neurons)
OT = N // P               # 8 output-neuron tiles
FH = 512                  # matmul moving free dim (one fp32 PSUM bank)
BD = BC * D               # 64
GRP = 4                   # o-tiles in the k-outer leading group
PCOLS = OT * DS + OT * D + OT * D + OT   # packed params: 1160

F32 = mybir.dt.float32
BF16 = mybir.dt.bfloat16

_NC_CACHE = {}


def legalize_waits(nc, max_attached=1):
    """Split multi-semaphore waits onto preceding same-engine NOPs.

    The walrus build in this environment accepts at most one sync-wait
    command per instruction (setupSyncWait: "Too many sync wait commands"),
    but Tile attaches one wait per out-of-date engine clock.  An engine is
    in-order, so hoisting the extra waits onto NOPs immediately before the
    instruction is semantics-preserving.
    """
    nid = 0
    for f in nc.m.functions:
        for blk in f.blocks:
            new = []
            changed = False
            for inst in blk.instructions:
                si = inst.sync_info
                if si is not None and si.on_wait and len(si.on_wait) > max_attached:
                    waits = list(si.on_wait)
                    for w in waits[:-max_attached]:
                        nid += 1
                        nop = mybir.InstNoOp(name=f"WSPLIT-{nid}", ins=[], outs=[])
                        nop.engine = inst.engine
                        nop.sync_info = mybir.SyncInfo(on_wait=[w], on_update=[])
                        new.append(nop)
                    inst.sync_info = mybir.SyncInfo(
                        on_wait=waits[-max_attached:], on_update=list(si.on_update)
                    )
                    changed = True
                new.append(inst)
            if changed:
                blk.instructions = new
    return nc


def build_nc(mm_dtype=BF16, legalize=True):
    """Build the single-core Bass program (SPMD: same program on all cores)."""
    nc = bass.Bass()
    mt = nc.declare_dram_parameter("mt", [N, N], mm_dtype, isOutput=False)
    xc = nc.declare_dram_parameter("xc", [N, BC * DS], mm_dtype, isOutput=False)
    params = nc.declare_dram_parameter("params", [P, PCOLS], F32, isOutput=False)
    out = nc.declare_dram_parameter("out", [P, OT * BC], F32, isOutput=True)

    AF = mybir.ActivationFunctionType
    AX = mybir.AxisListType
    OP = mybir.AluOpType

    with tile.TileContext(nc) as tc, ExitStack() as ctx:
        wpool = ctx.enter_context(tc.tile_pool(name="weights", bufs=1))
        xpool = ctx.enter_context(tc.tile_pool(name="xin", bufs=1))
        pspool = ctx.enter_context(tc.tile_pool(name="ps", bufs=8, space="PSUM"))
        prpool = ctx.enter_context(tc.tile_pool(name="prp", bufs=3))
        smpool = ctx.enter_context(tc.tile_pool(name="smp", bufs=3))

        # --- PE pre-warm: dummy matmuls on memset scratch while the first
        # input chunk is still in flight.  The HAM clock gate needs ~3.4us
        # of sustained PE activity to lift the PE from 1.2 to 2.4 GHz;
        # warming during the DMA wait means the real matmuls run at full
        # rate from the start.  Placed first so the memset precedes the
        # DMA issue on GpSimd and the dummies start right after the PE's
        # preamble. ---
        warm_sb = wpool.tile([P, FH], BF16, tag="warm", name="warm_sb")
        nc.gpsimd.memset(warm_sb[:], 0.0)
        warm_ps = pspool.tile([P, FH], F32, tag="ps", name="warm_ps")
        for _ in range(8):
            nc.tensor.matmul(
                warm_ps[:], lhsT=warm_sb[:, 0:P], rhs=warm_sb[:],
                start=True, stop=True,
            )

        # --- input DMAs: x chunks on Sync, mt chunks on Scalar (parallel
        # HWDGE issue); whole [128, 2KB-row] chunks for full DMA rate. ---
        x_tiles, mt_tiles = [], []
        x0_dma = None
        for k in range(KT):
            xt = xpool.tile([P, BC * DS], mm_dtype, tag=f"x{k}", name=f"x{k}")
            mtk = xpool.tile([P, N], mm_dtype, tag=f"m{k}", name=f"m{k}")
            xdma = nc.sync.dma_start(xt[:], xc[k * P:(k + 1) * P, :])
            if k == 0:
                x0_dma = xdma
            nc.scalar.dma_start(mtk[:], mt[k * P:(k + 1) * P, :])
            x_tiles.append(xt)
            mt_tiles.append(mtk)

        # Delay the (non-critical) params transfer behind the first x chunk
        # so it doesn't steal HBM bandwidth from the matmul-critical loads.
        params_sb = wpool.tile([P, PCOLS], F32, tag="params", name="params_sb")
        pdma = nc.gpsimd.dma_start(params_sb[:], params[:, :])
        from bass_rust import add_dep_helper
        add_dep_helper(pdma.ins, x0_dma.ins, sync=True,
                       reason="params after critical first chunk")
        W0, B0, W1, B1 = 0, OT * DS, OT * DS + OT * D, OT * DS + 2 * OT * D

        out_sb = wpool.tile([P, OT * BC], F32, tag="out", name="out_sb")

        def postprocess(t, pst, latency_opt=False, gps_heavy=False):
            # prod[o, b, (d,s)] = xm * w_syn (broadcast over b), read
            # straight from PSUM per half (fp32, 1x DVE).
            prod = prpool.tile([P, BC * DS], F32, tag="prod", name=f"prod{t}")
            for h in range(2):
                nc.vector.tensor_mul(
                    prod[:, h * FH:(h + 1) * FH].rearrange(
                        "p (b q) -> p b q", b=BC // 2),
                    pst[h][:].rearrange("p (b q) -> p b q", b=BC // 2),
                    params_sb[:, W0 + t * DS:W0 + (t + 1) * DS].unsqueeze(1)
                    .broadcast_to([P, BC // 2, DS]),
                )
            # Big s-reduce stays on DVE, contiguous with the mults so the
            # in-order DVE stream never stalls on another engine mid-tile.
            dp = smpool.tile([P, BD], F32, tag="dp", name=f"dp{t}")
            pv = prod[:].rearrange("p (bd s) -> p bd s", s=S)
            if gps_heavy:
                # Whole s-reduce as a GpSimd pairwise tree: frees the
                # in-order DVE right after the PSUM mults so the LAST
                # tile's latency chain is not blocked behind this one.
                gr1 = smpool.tile([P, BD * 8], F32, tag="gr1", name=f"gr1{t}")
                nc.gpsimd.tensor_add(
                    gr1[:].rearrange("p (bd s) -> p bd s", s=8),
                    pv[:, :, 0:8], pv[:, :, 8:16],
                )
                g1v = gr1[:].rearrange("p (bd s) -> p bd s", s=8)
                gr2 = smpool.tile([P, BD * 4], F32, tag="gr2", name=f"gr2{t}")
                nc.gpsimd.tensor_add(
                    gr2[:].rearrange("p (bd s) -> p bd s", s=4),
                    g1v[:, :, 0:4], g1v[:, :, 4:8],
                )
                g2v = gr2[:].rearrange("p (bd s) -> p bd s", s=4)
                gr3 = smpool.tile([P, BD * 2], F32, tag="gr3", name=f"gr3{t}")
                nc.gpsimd.tensor_add(
                    gr3[:].rearrange("p (bd s) -> p bd s", s=2),
                    g2v[:, :, 0:2], g2v[:, :, 2:4],
                )
                g3v = gr3[:].rearrange("p (bd s) -> p bd s", s=2)
                nc.gpsimd.tensor_add(
                    dp[:].unsqueeze(2), g3v[:, :, 0:1], g3v[:, :, 1:2],
                )
            else:
                nc.vector.tensor_reduce(dp[:], pv, axis=AX.X, op=OP.add)
            bias_eng = nc.vector if latency_opt else nc.gpsimd
            bias_eng.tensor_add(
                dp[:].rearrange("p (b d) -> p b d", d=D),
                dp[:].rearrange("p (b d) -> p b d", d=D),
                params_sb[:, B0 + t * D:B0 + (t + 1) * D].unsqueeze(1)
                .broadcast_to([P, BC, D]),
            )
            dend = smpool.tile([P, BD], F32, tag="dend", name=f"dend{t}")
            nc.scalar.activation(dend[:], dp[:], AF.Tanh)
            # soma: * w_dend, reduce over d, sigmoid(+b_soma)
            sp = smpool.tile([P, BD], F32, tag="sp", name=f"sp{t}")
            soma = smpool.tile([P, BC], F32, tag="soma", name=f"soma{t}")
            if latency_opt:
                nc.vector.tensor_mul(
                    sp[:].rearrange("p (b d) -> p b d", d=D),
                    dend[:].rearrange("p (b d) -> p b d", d=D),
                    params_sb[:, W1 + t * D:W1 + (t + 1) * D].unsqueeze(1)
                    .broadcast_to([P, BC, D]),
                )
                nc.vector.tensor_reduce(
                    soma[:],
                    sp[:].rearrange("p (b d) -> p b d", d=D),
                    axis=AX.X,
                    op=OP.add,
                )
            else:
                # Soma stage entirely on GpSimd (mult + pairwise d-tree),
                # keeping the DVE stream free for the next tile's mults.
                nc.gpsimd.tensor_mul(
                    sp[:].rearrange("p (b d) -> p b d", d=D),
                    dend[:].rearrange("p (b d) -> p b d", d=D),
                    params_sb[:, W1 + t * D:W1 + (t + 1) * D].unsqueeze(1)
                    .broadcast_to([P, BC, D]),
                )
                r1 = smpool.tile([P, BC * 4], F32, tag="r1", name=f"r1{t}")
                spv = sp[:].rearrange("p (b d) -> p b d", d=D)
                nc.gpsimd.tensor_add(
                    r1[:].rearrange("p (b d) -> p b d", d=4),
                    spv[:, :, 0:4], spv[:, :, 4:8],
                )
                r2 = smpool.tile([P, BC * 2], F32, tag="r2", name=f"r2{t}")
                r1v = r1[:].rearrange("p (b d) -> p b d", d=4)
                nc.gpsimd.tensor_add(
                    r2[:].rearrange("p (b d) -> p b d", d=2),
                    r1v[:, :, 0:2], r1v[:, :, 2:4],
                )
                r2v = r2[:].rearrange("p (b d) -> p b d", d=2)
                nc.gpsimd.tensor_add(
                    soma[:].unsqueeze(2), r2v[:, :, 0:1], r2v[:, :, 1:2],
                )
            nc.scalar.activation(
                out_sb[:, t * BC:(t + 1) * BC], soma[:], AF.Sigmoid,
                bias=params_sb[:, B1 + t:B1 + t + 1],
            )

        def mm(pst, t, k):
            for h in range(2):
                nc.tensor.matmul(
                    pst[h][:],
                    lhsT=mt_tiles[k][:, t * P:(t + 1) * P],
                    rhs=x_tiles[k][:, h * FH:(h + 1) * FH],
                    start=(k == 0),
                    stop=(k == KT - 1),
                )

        # Leading group: k-outer over o-tiles 0..GRP-1 — per-k PE work
        # (GRP matmul pairs) paces with the per-k chunk DMA.
        pst = {}
        for t in range(GRP):
            pst[t] = [
                pspool.tile([P, FH], F32, tag="ps", name=f"ps{t}_{h}")
                for h in range(2)
            ]
        for k in range(KT):
            for t in range(GRP):
                mm(pst[t], t, k)
        for t in range(GRP):
            postprocess(t, pst[t])

        # Remaining o-tiles: one at a time, k-inner; each tile's chain
        # overlaps the next tile's matmuls.
        for t in range(GRP, OT):
            pstt = [
                pspool.tile([P, FH], F32, tag="ps", name=f"ps{t}_{h}")
                for h in range(2)
            ]
            for k in range(KT):
                mm(pstt, t, k)
            postprocess(t, pstt, latency_opt=(t == OT - 1))

        nc.scalar.dma_start(out[:, :], out_sb[:])

    if legalize:
        legalize_waits(nc)
    return nc


def get_nc(mm_dtype=BF16):
    key = str(mm_dtype)
    if key not in _NC_CACHE:
        _NC_CACHE[key] = build_nc(mm_dtype)
    return _NC_CACHE[key]


def pack_params(w_syn, b_dend, w_dend, b_soma):
    """Pack per-neuron parameters into one [128, 1160] fp32 array whose
    columns match the SBUF params tile layout (w_syn | b_dend | w_dend |
    b_soma, each o-tile-major)."""
    ws = np.asarray(w_syn, np.float32).reshape(OT, P, DS).transpose(1, 0, 2).reshape(P, OT * DS)
    bd = np.asarray(b_dend, np.float32).reshape(OT, P, D).transpose(1, 0, 2).reshape(P, OT * D)
    wd = np.asarray(w_dend, np.float32).reshape(OT, P, D).transpose(1, 0, 2).reshape(P, OT * D)
    bs = np.asarray(b_soma, np.float32).reshape(OT, P).T
    return np.ascontiguousarray(np.concatenate([ws, bd, wd, bs], axis=1))


def prepare_in_maps(x, matriz_conexao, w_syn, b_dend, w_dend, b_soma,
                    mm_np_dtype=ml_dtypes.bfloat16):
    x = np.asarray(x, dtype=np.float32)
    mt_np = np.ascontiguousarray(np.asarray(matriz_conexao, np.float32).T).astype(mm_np_dtype)
    params_np = pack_params(w_syn, b_dend, w_dend, b_soma)
    xt = np.ascontiguousarray(x.transpose(1, 0, 2, 3).reshape(N, B, DS))
    in_maps = []
    for c in range(NCORES):
        xc_np = np.ascontiguousarray(
            xt[:, c * BC:(c + 1) * BC, :].reshape(N, BC * DS)
        ).astype(mm_np_dtype)
        in_maps.append({"mt": mt_np, "xc": xc_np, "params": params_np})
    return in_maps


def assemble_output(results):
    outs = []
    for c in range(NCORES):
        oc = np.asarray(results[c]["out"])          # [P, OT*BC] = (oi, (t, b))
        outs.append(oc.reshape(P, OT, BC).transpose(2, 1, 0).reshape(BC, N))
    return np.ascontiguousarray(np.concatenate(outs, axis=0).astype(np.float32))


def kernel(x, matriz_conexao, w_syn, b_dend, w_dend, b_soma):
    from concourse.bass_utils import run_bass_kernel_spmd
    in_maps = prepare_in_maps(x, matriz_conexao, w_syn, b_dend, w_dend, b_soma)
    nc = get_nc()
    res = run_bass_kernel_spmd(nc, in_maps, list(range(NCORES)))
    return assemble_output(res.results)



# revision 8
# speedup vs baseline: 1.2669x; 1.2669x over previous
"""Trainium2 Bass kernel for nn_Camada_33612414059004.

Computes, for x:[B,N,D,S], M:[N,N], w_syn:[N,D,S], b_dend:[N,D],
w_dend:[N,D], b_soma:[N]:

    xm    = einsum('bids,oi->bods', x, M)
    dend  = tanh(einsum('bnds,nds->bnd', xm, w_syn) + b_dend)
    soma  = einsum('bnd,nd->bn', dend, w_dend) + b_soma
    out   = sigmoid(soma)                                  # [B, N]

Sharding: data-parallel over batch across 8 NeuronCores (B=64 -> 8/core),
zero cross-core communication.  Per core the dominant work is the
connectivity matmul  M[o,i] @ x[i, (b,d,s)]  ([1024x1024]x[1024x1024],
2.15 GFLOP, bf16 operands / fp32 PSUM accumulate) on the TensorEngine.
The per-neuron stages run in fp32, spread so no engine exceeds the PE's
3.5us-per-o-tile matmul time: Vector multiplies the PSUM result by w_syn
and does the s/d reductions, GpSimd does the small bias-add and w_dend
multiply, Scalar does tanh / sigmoid(+b_soma).

Per-core on-chip layout: output neurons `o` on the 128 SBUF partitions
(8 o-tiles), free dim ordered (b, d, s).

Schedule: o-tiles 0-3 accumulate k-outer — per-k PE work (4 matmul pairs)
matches the per-k input DMA time, riding out the input stream; o-tiles
4-7 run k-inner one at a time, each tile's postprocess chain pipelining
against the next tile's matmuls.  x chunks load on the Sync HWDGE, mt
chunks on the Scalar HWDGE (parallel issue); all per-neuron parameters
are packed host-side into one contiguous [128, 1160] fp32 array moved by
a single DMA.
"""

import numpy as np
import ml_dtypes
from contextlib import ExitStack

import concourse.bass as bass
import concourse.mybir as mybir
import concourse.tile as tile

B, N, D, S = 64, 1024, 8, 16
NCORES = 8
BC = B // NCORES          # batches per core = 8
DS = D * S                # 128
P = 128                   # SBUF partitions
KT = N // P               # 8 contraction chunks (input neurons)
KT2 = KT // 2             # 4 DoubleRow chunk pairs (K=256 each)
OT = N // P               # 8 output-neuron tiles
FH = 512                  # matmul moving free dim (one fp32 PSUM bank)
BD = BC * D               # 64
GRP = 4                   # o-tiles in the k-outer leading group
PCOLS = OT * DS + OT * D + OT * D + OT   # packed params: 1160

F32 = mybir.dt.float32
BF16 = mybir.dt.bfloat16
FP8 = mybir.dt.float8e4

_NC_CACHE = {}


def legalize_waits(nc, max_attached=1):
    """Split multi-semaphore waits onto preceding same-engine NOPs.

    The walrus build in this environment accepts at most one sync-wait
    command per instruction (setupSyncWait: "Too many sync wait commands"),
    but Tile attaches one wait per out-of-date engine clock.  An engine is
    in-order, so hoisting the extra waits onto NOPs immediately before the
    instruction is semantics-preserving.
    """
    nid = 0
    for f in nc.m.functions:
        for blk in f.blocks:
            new = []
            changed = False
            for inst in blk.instructions:
                si = inst.sync_info
                if si is not None and si.on_wait and len(si.on_wait) > max_attached:
                    waits = list(si.on_wait)
                    for w in waits[:-max_attached]:
                        nid += 1
                        nop = mybir.InstNoOp(name=f"WSPLIT-{nid}", ins=[], outs=[])
                        nop.engine = inst.engine
                        nop.sync_info = mybir.SyncInfo(on_wait=[w], on_update=[])
                        new.append(nop)
                    inst.sync_info = mybir.SyncInfo(
                        on_wait=waits[-max_attached:], on_update=list(si.on_update)
                    )
                    changed = True
                new.append(inst)
            if changed:
                blk.instructions = new
    return nc


def build_nc(mm_dtype=FP8, legalize=True):
    """Build the single-core Bass program (SPMD: same program on all cores)."""
    nc = bass.Bass()
    mt = nc.declare_dram_parameter("mt", [N, N], mm_dtype, isOutput=False)
    xc = nc.declare_dram_parameter("xc", [N, BC * DS], mm_dtype, isOutput=False)
    params = nc.declare_dram_parameter("params", [P, PCOLS], F32, isOutput=False)
    out = nc.declare_dram_parameter("out", [P, OT * BC], F32, isOutput=True)

    AF = mybir.ActivationFunctionType
    AX = mybir.AxisListType
    OP = mybir.AluOpType

    with tile.TileContext(nc) as tc, ExitStack() as ctx:
        wpool = ctx.enter_context(tc.tile_pool(name="weights", bufs=1))
        xpool = ctx.enter_context(tc.tile_pool(name="xin", bufs=1))
        pspool = ctx.enter_context(tc.tile_pool(name="ps", bufs=8, space="PSUM"))
        prpool = ctx.enter_context(tc.tile_pool(name="prp", bufs=3))
        smpool = ctx.enter_context(tc.tile_pool(name="smp", bufs=3))

        # --- PE pre-warm: dummy matmuls on memset scratch while the first
        # input chunk is still in flight.  The HAM clock gate needs ~3.4us
        # of sustained PE activity to lift the PE from 1.2 to 2.4 GHz;
        # warming during the DMA wait means the real matmuls run at full
        # rate from the start.  Placed first so the memset precedes the
        # DMA issue on GpSimd and the dummies start right after the PE's
        # preamble. ---
        warm_sb = wpool.tile([P, FH], BF16, tag="warm", name="warm_sb")
        nc.gpsimd.memset(warm_sb[:], 0.0)
        warm_ps = pspool.tile([P, FH], F32, tag="ps", name="warm_ps")
        for _ in range(8):
            nc.tensor.matmul(
                warm_ps[:], lhsT=warm_sb[:, 0:P], rhs=warm_sb[:],
                start=True, stop=True,
            )

        # --- input DMAs: per DoubleRow chunk-pair [128, 2, cols] tiles
        # (contraction row g*128+p), x on Sync, mt on Scalar (parallel
        # HWDGE issue). ---
        x_tiles, mt_tiles = [], []
        x0_dma = None
        for k in range(KT2):
            xt = xpool.tile([P, 2 * BC * DS], mm_dtype, tag=f"x{k}", name=f"x{k}")
            mtk = xpool.tile([P, 2 * N], mm_dtype, tag=f"m{k}", name=f"m{k}")
            xdma = nc.sync.dma_start(
                xt[:].rearrange("p (g c) -> p g c", g=2),
                xc[k * 2 * P:(k + 1) * 2 * P, :].rearrange(
                    "(g p) c -> p g c", g=2),
            )
            if k == 0:
                x0_dma = xdma
            nc.scalar.dma_start(
                mtk[:].rearrange("p (g c) -> p g c", g=2),
                mt[k * 2 * P:(k + 1) * 2 * P, :].rearrange(
                    "(g p) c -> p g c", g=2),
            )
            x_tiles.append(xt)
            mt_tiles.append(mtk)

        # Delay the (non-critical) params transfer behind the first x chunk
        # so it doesn't steal HBM bandwidth from the matmul-critical loads.
        params_sb = wpool.tile([P, PCOLS], F32, tag="params", name="params_sb")
        pdma = nc.gpsimd.dma_start(params_sb[:], params[:, :])
        from bass_rust import add_dep_helper
        add_dep_helper(pdma.ins, x0_dma.ins, sync=True,
                       reason="params after critical first chunk")
        W0, B0, W1, B1 = 0, OT * DS, OT * DS + OT * D, OT * DS + 2 * OT * D

        out_sb = wpool.tile([P, OT * BC], F32, tag="out", name="out_sb")

        def postprocess(t, pst, latency_opt=False, gps_heavy=False):
            # prod[o, b, (d,s)] = xm * w_syn (broadcast over b), read
            # straight from PSUM per half (fp32, 1x DVE).
            prod = prpool.tile([P, BC * DS], F32, tag="prod", name=f"prod{t}")
            for h in range(2):
                nc.vector.tensor_mul(
                    prod[:, h * FH:(h + 1) * FH].rearrange(
                        "p (b q) -> p b q", b=BC // 2),
                    pst[h][:].rearrange("p (b q) -> p b q", b=BC // 2),
                    params_sb[:, W0 + t * DS:W0 + (t + 1) * DS].unsqueeze(1)
                    .broadcast_to([P, BC // 2, DS]),
                )
            # Big s-reduce stays on DVE, contiguous with the mults so the
            # in-order DVE stream never stalls on another engine mid-tile.
            dp = smpool.tile([P, BD], F32, tag="dp", name=f"dp{t}")
            pv = prod[:].rearrange("p (bd s) -> p bd s", s=S)
            if gps_heavy:
                # Whole s-reduce as a GpSimd pairwise tree: frees the
                # in-order DVE right after the PSUM mults so the LAST
                # tile's latency chain is not blocked behind this one.
                gr1 = smpool.tile([P, BD * 8], F32, tag="gr1", name=f"gr1{t}")
                nc.gpsimd.tensor_add(
                    gr1[:].rearrange("p (bd s) -> p bd s", s=8),
                    pv[:, :, 0:8], pv[:, :, 8:16],
                )
                g1v = gr1[:].rearrange("p (bd s) -> p bd s", s=8)
                gr2 = smpool.tile([P, BD * 4], F32, tag="gr2", name=f"gr2{t}")
                nc.gpsimd.tensor_add(
                    gr2[:].rearrange("p (bd s) -> p bd s", s=4),
                    g1v[:, :, 0:4], g1v[:, :, 4:8],
                )
                g2v = gr2[:].rearrange("p (bd s) -> p bd s", s=4)
                gr3 = smpool.tile([P, BD * 2], F32, tag="gr3", name=f"gr3{t}")
                nc.gpsimd.tensor_add(
                    gr3[:].rearrange("p (bd s) -> p bd s", s=2),
                    g2v[:, :, 0:2], g2v[:, :, 2:4],
                )
                g3v = gr3[:].rearrange("p (bd s) -> p bd s", s=2)
                nc.gpsimd.tensor_add(
                    dp[:].unsqueeze(2), g3v[:, :, 0:1], g3v[:, :, 1:2],
                )
            else:
                nc.vector.tensor_reduce(dp[:], pv, axis=AX.X, op=OP.add)
            bias_eng = nc.vector if latency_opt else nc.gpsimd
            bias_eng.tensor_add(
                dp[:].rearrange("p (b d) -> p b d", d=D),
                dp[:].rearrange("p (b d) -> p b d", d=D),
                params_sb[:, B0 + t * D:B0 + (t + 1) * D].unsqueeze(1)
                .broadcast_to([P, BC, D]),
            )
            dend = smpool.tile([P, BD], F32, tag="dend", name=f"dend{t}")
            nc.scalar.activation(dend[:], dp[:], AF.Tanh)
            # soma: * w_dend, reduce over d, sigmoid(+b_soma)
            sp = smpool.tile([P, BD], F32, tag="sp", name=f"sp{t}")
            soma = smpool.tile([P, BC], F32, tag="soma", name=f"soma{t}")
            if latency_opt:
                nc.vector.tensor_mul(
                    sp[:].rearrange("p (b d) -> p b d", d=D),
                    dend[:].rearrange("p (b d) -> p b d", d=D),
                    params_sb[:, W1 + t * D:W1 + (t + 1) * D].unsqueeze(1)
                    .broadcast_to([P, BC, D]),
                )
                nc.vector.tensor_reduce(
                    soma[:],
                    sp[:].rearrange("p (b d) -> p b d", d=D),
                    axis=AX.X,
                    op=OP.add,
                )
            else:
                # Soma stage entirely on GpSimd (mult + pairwise d-tree),
                # keeping the DVE stream free for the next tile's mults.
                nc.gpsimd.tensor_mul(
                    sp[:].rearrange("p (b d) -> p b d", d=D),
                    dend[:].rearrange("p (b d) -> p b d", d=D),
                    params_sb[:, W1 + t * D:W1 + (t + 1) * D].unsqueeze(1)
                    .broadcast_to([P, BC, D]),
                )
                r1 = smpool.tile([P, BC * 4], F32, tag="r1", name=f"r1{t}")
                spv = sp[:].rearrange("p (b d) -> p b d", d=D)
                nc.gpsimd.tensor_add(
                    r1[:].rearrange("p (b d) -> p b d", d=4),
                    spv[:, :, 0:4], spv[:, :, 4:8],
                )
                r2 = smpool.tile([P, BC * 2], F32, tag="r2", name=f"r2{t}")
                r1v = r1[:].rearrange("p (b d) -> p b d", d=4)
                nc.gpsimd.tensor_add(
                    r2[:].rearrange("p (b d) -> p b d", d=2),
                    r1v[:, :, 0:2], r1v[:, :, 2:4],
                )
                r2v = r2[:].rearrange("p (b d) -> p b d", d=2)
                nc.gpsimd.tensor_add(
                    soma[:].unsqueeze(2), r2v[:, :, 0:1], r2v[:, :, 1:2],
                )
            nc.scalar.activation(
                out_sb[:, t * BC:(t + 1) * BC], soma[:], AF.Sigmoid,
                bias=params_sb[:, B1 + t:B1 + t + 1],
            )

        def mm(pst, t, k):
            mtv = mt_tiles[k][:].rearrange("p (g c) -> p g c", g=2)
            xv = x_tiles[k][:].rearrange("p (g c) -> p g c", g=2)
            for h in range(2):
                nc.tensor.matmul(
                    pst[h][:],
                    lhsT=mtv[:, :, t * P:(t + 1) * P],
                    rhs=xv[:, :, h * FH:(h + 1) * FH],
                    start=(k == 0),
                    stop=(k == KT2 - 1),
                    perf_mode=mybir.MatmulPerfMode.DoubleRow,
                )

        # Leading group: k-outer over o-tiles 0..GRP-1 — per-k PE work
        # (GRP matmul pairs) paces with the per-k chunk DMA.
        pst = {}
        for t in range(GRP):
            pst[t] = [
                pspool.tile([P, FH], F32, tag="ps", name=f"ps{t}_{h}")
                for h in range(2)
            ]
        for k in range(KT2):
            for t in range(GRP):
                mm(pst[t], t, k)
        for t in range(GRP):
            postprocess(t, pst[t])

        # Remaining o-tiles: one at a time, k-inner; each tile's chain
        # overlaps the next tile's matmuls.
        for t in range(GRP, OT):
            pstt = [
                pspool.tile([P, FH], F32, tag="ps", name=f"ps{t}_{h}")
                for h in range(2)
            ]
            for k in range(KT2):
                mm(pstt, t, k)
            postprocess(t, pstt, latency_opt=(t == OT - 1))

        nc.scalar.dma_start(out[:, :], out_sb[:])

    if legalize:
        legalize_waits(nc)
    return nc


def get_nc(mm_dtype=FP8):
    key = str(mm_dtype)
    if key not in _NC_CACHE:
        _NC_CACHE[key] = build_nc(mm_dtype)
    return _NC_CACHE[key]


def pack_params(w_syn, b_dend, w_dend, b_soma):
    """Pack per-neuron parameters into one [128, 1160] fp32 array whose
    columns match the SBUF params tile layout (w_syn | b_dend | w_dend |
    b_soma, each o-tile-major)."""
    ws = np.asarray(w_syn, np.float32).reshape(OT, P, DS).transpose(1, 0, 2).reshape(P, OT * DS)
    bd = np.asarray(b_dend, np.float32).reshape(OT, P, D).transpose(1, 0, 2).reshape(P, OT * D)
    wd = np.asarray(w_dend, np.float32).reshape(OT, P, D).transpose(1, 0, 2).reshape(P, OT * D)
    bs = np.asarray(b_soma, np.float32).reshape(OT, P).T
    return np.ascontiguousarray(np.concatenate([ws, bd, wd, bs], axis=1))


def prepare_in_maps(x, matriz_conexao, w_syn, b_dend, w_dend, b_soma,
                    mm_np_dtype=ml_dtypes.float8_e4m3):
    x = np.asarray(x, dtype=np.float32)
    mt_np = np.ascontiguousarray(np.asarray(matriz_conexao, np.float32).T).astype(mm_np_dtype)
    params_np = pack_params(w_syn, b_dend, w_dend, b_soma)
    xt = np.ascontiguousarray(x.transpose(1, 0, 2, 3).reshape(N, B, DS))
    in_maps = []
    for c in range(NCORES):
        xc_np = np.ascontiguousarray(
            xt[:, c * BC:(c + 1) * BC, :].reshape(N, BC * DS)
        ).astype(mm_np_dtype)
        in_maps.append({"mt": mt_np, "xc": xc_np, "params": params_np})
    return in_maps


def assemble_output(results):
    outs = []
    for c in range(NCORES):
        oc = np.asarray(results[c]["out"])          # [P, OT*BC] = (oi, (t, b))
        outs.append(oc.reshape(P, OT, BC).transpose(2, 1, 0).reshape(BC, N))
    return np.ascontiguousarray(np.concatenate(outs, axis=0).astype(np.float32))


def kernel(x, matriz_conexao, w_syn, b_dend, w_dend, b_soma):
    from concourse.bass_utils import run_bass_kernel_spmd
    in_maps = prepare_in_maps(x, matriz_conexao, w_syn, b_dend, w_dend, b_soma)
    nc = get_nc()
    res = run_bass_kernel_spmd(nc, in_maps, list(range(NCORES)))
    return assemble_output(res.results)

